# revision 25
# baseline (speedup 1.0000x reference)
"""GAT (2-layer) + edge MLP predictor on 8 TRN2 NeuronCores — v3.

Design (edge/1D graph partition parallelism, dst-aligned column-major):
  - dst nodes ranked by in-degree (desc), dealt round-robin to cores;
    bands of 1024 ranks form one tile across all 8 cores, so per-tile
    max degree is uniform and shared (SPMD single program).
  - Edge slots are COLUMN-MAJOR: slot (p, c) of a tile is the c-th edge
    of dst node p.  dst == partition, so softmax denominators are
    free-axis reductions.
  - LAYER 1 does not gather at all: the host pre-permutes the INPUT
    node features into edge-slot order (XeT stream) plus per-slot
    attention logits el1 and per-node er1, so layer-1 edge rows are
    produced by plain matmuls against W1 on a linear DMA stream.
  - LAYER 2 + SCORE gather per-edge rows from device-built tables with
    InstDMAGatherAnt.  Desc-gen is the bottleneck (one Q7 core-pair per
    SWDGE queue), so calls round-robin across 4 queues and are issued
    with deep lookahead so 4 desc-gens run concurrently.
  - The 50176-row tables exceed int16, so three overlapping 32768-row
    windows (A/M/B) are used; per-node edges are split between windows
    by a small per-tile LP.  Padding slots gather a special row with
    el = -200 so exp() zeroes them (no masks).
  - Layer-2 table and score table are assembled via chunked AllGather
    interleaved with compute.
  - Score: score_e = hs[src_e] (gathered) + hd[dst_e] (local).
"""

import math

import numpy as np

# ---- problem constants ----
N = 50000
E = 800000
FIN = 128
H = 3
D = 64
F = 192
CLS = 10
NEG = 0.2
NCORES = 8
NPC = 6250
TPC = 49
ROWE = 256          # table row elems (bf16)
NSL = 6272          # padded nodes per core (49 tiles)
NTAB = NCORES * NSL                     # 50176 = 392*128
WIN = 32768
W_OFF = (0, 8704, NTAB - WIN)           # A, M, B window bases
SPEC = 3 * NSL + 6250                   # special row: core3 pad slot,
                                        # inside the A&M&B overlap
PAD_IDX = (SPEC, SPEC - W_OFF[1], SPEC - W_OFF[2])
EL_NEG = -200.0
GCH = 12            # gather chunks per call (1536 idx)
GXE = 16            # xe stream chunks per DMA call
LA_CH = 24          # gather lookahead chunks per window
NQ = 4              # SWDGE queues

_COMPILED = {}


# ======================================================================
# host-side preprocessing
# ======================================================================

def _prep(src, dst, nfeats, W1, al1, ar1, b1, W2, al2, ar2, b2, Wp, bp):
    import ml_dtypes
    bf16 = ml_dtypes.bfloat16
    src = np.asarray(src).astype(np.int64)
    dst = np.asarray(dst).astype(np.int64)

    deg = np.bincount(dst, minlength=N)
    assert deg.min() >= 1, "0-degree dst nodes unsupported"
    order = np.argsort(-deg, kind='stable')
    rank = np.empty(N, np.int64)
    rank[order] = np.arange(N)
    core = rank % NCORES
    q = rank // NCORES
    tpos = core * NSL + q

    e_core = core[dst]
    e_tile = (q // 128)[dst]
    e_row = (q % 128)[dst]
    e_tpos = tpos[src]

    # segments: 0 A-only, 1 AM, 2 AMB, 3 MB, 4 B-only
    WM, WB = W_OFF[1], W_OFF[2]
    seg = np.zeros(E, np.int8)
    seg[(e_tpos >= WM) & (e_tpos < WB)] = 1
    seg[(e_tpos >= WB) & (e_tpos < WIN)] = 2
    seg[(e_tpos >= WIN) & (e_tpos < WM + WIN)] = 3
    seg[e_tpos >= WM + WIN] = 4

    key = (e_core * TPC + e_tile) * 128 + e_row
    nk = NCORES * TPC * 128
    cnt = np.zeros((5, nk), np.int64)
    for s in range(5):
        cnt[s] = np.bincount(key[seg == s], minlength=nk)
    cnt = cnt.reshape(5, NCORES, TPC, 128)

    CW = np.zeros((3, TPC), np.int64)          # caps per window per tile
    for t in range(TPC):
        aa = cnt[0, :, t, :].ravel()
        am = cnt[1, :, t, :].ravel()
        amb = cnt[2, :, t, :].ravel()
        mb = cnt[3, :, t, :].ravel()
        bb = cnt[4, :, t, :].ravel()
        d = aa + am + amb + mb + bb
        c1 = int(aa.max()); c2 = int(bb.max())
        c3 = int((aa + am).max()); c4 = int((bb + mb).max())
        c5 = int(d.max()); c6 = int((aa + bb).max())
        best = None
        for ca in range(c1, c5 + 2):
            for cm in range(max(0, c3 - ca), c5 + 2 - ca):
                cb = max(c2, c4 - cm, c5 - ca - cm, c6 - ca, 0)
                cost = ca + cm + cb
                if best is None or cost < best[0]:
                    best = (cost, ca, cm, cb)
        CW[0, t], CW[1, t], CW[2, t] = best[1:]

    sW = np.zeros((3, TPC), np.int64)          # chunk start per window
    sW[:, 1:] = np.cumsum(CW[:, :-1], axis=1)
    TC = CW.sum(axis=1)                        # total chunks per window

    idx16 = [np.full((NCORES, int(TC[w]) * 128), PAD_IDX[w], np.int64)
             for w in range(3)]
    orig = [np.full((NCORES, int(TC[w]) * 128), -1, np.int64)
            for w in range(3)]

    sort = np.argsort(key, kind='stable')
    bnd = np.searchsorted(key[sort], np.arange(nk + 1))
    for k in range(NCORES):
        for t in range(TPC):
            ca, cm, cb = CW[0, t], CW[1, t], CW[2, t]
            for p in range(128):
                b0 = bnd[(k * TPC + t) * 128 + p]
                b1_ = bnd[(k * TPC + t) * 128 + p + 1]
                eids = sort[b0:b1_]
                sg = seg[eids]
                l0 = list(eids[sg == 0]); l1 = list(eids[sg == 1])
                l2 = list(eids[sg == 2]); l3 = list(eids[sg == 3])
                l4 = list(eids[sg == 4])
                la = l0
                take = min(ca - len(la), len(l1))
                la = la + l1[:take]; rem1 = l1[take:]
                take = min(ca - len(la), len(l2))
                la = la + l2[:take]; rem2 = l2[take:]
                lm = rem1
                take = min(cm - len(lm), len(rem2))
                lm = lm + rem2[:take]; rem2 = rem2[take:]
                take = min(cm - len(lm), len(l3))
                lm = lm + l3[:take]
                lb = l3[take:] + l4 + rem2
                assert len(la) <= ca and len(lm) <= cm and len(lb) <= cb
                for w, lst in ((0, la), (1, lm), (2, lb)):
                    for c, e in enumerate(lst):
                        pos = (sW[w, t] + c) * 128 + p
                        idx16[w][k, pos] = e_tpos[e] - W_OFF[w]
                        orig[w][k, pos] = e

    def wrap16(a):
        n = a.shape[1]
        w = a.reshape(NCORES, n // 16, 16).transpose(0, 2, 1)
        return np.ascontiguousarray(np.tile(w, (1, 8, 1))).astype(np.int16)

    idxw = [wrap16(x) for x in idx16]

    # ---- weights ----
    def aug_w(W, al, ar):
        fin = W.shape[0]
        Wg = np.zeros((fin, ROWE), np.float32)
        Wg[:, :F] = W
        for h in range(H):
            Wg[:, F + h] = W[:, h * D:(h + 1) * D] @ al[h]
            Wg[:, F + 3 + h] = W[:, h * D:(h + 1) * D] @ ar[h]
        return Wg

    W1aug = aug_w(np.asarray(W1, np.float32), al1, ar1)
    W2aug = aug_w(np.asarray(W2, np.float32), al2, ar2)

    X32 = np.asarray(nfeats, np.float32)
    el1_n = X32 @ W1aug[:, F:F + 3]            # [N, 3]
    er1_n = X32 @ W1aug[:, F + 3:F + 6]        # [N, 3]
    XT_bf = np.ascontiguousarray(X32.T).astype(bf16)   # [FIN, N]

    # ---- per-tile chunk bookkeeping (host copies of device maps) ----
    NCHt = CW.sum(axis=0)
    sT = np.zeros(TPC, np.int64)
    sT[1:] = np.cumsum(NCHt[:-1])
    TOTCH = int(NCHt.sum())

    # ---- layer-1 edge-order streams ----
    XeT = np.zeros((NCORES, FIN, TOTCH * 128), bf16)
    el1e = np.full((NCORES, 128, TOTCH * 3), EL_NEG, np.float32)
    for k in range(NCORES):
        for t in range(TPC):
            g0 = sT[t]
            for w in range(3):
                cw = int(CW[w, t])
                if cw == 0:
                    continue
                lo = int(sW[w, t])
                blk = orig[w][k][lo * 128:(lo + cw) * 128]   # (c,p) order
                m = blk >= 0
                srcs = np.zeros(cw * 128, np.int64)
                srcs[m] = src[blk[m]]
                xb = XT_bf[:, srcs].copy()
                xb[:, ~m] = 0
                XeT[k][:, g0 * 128:(g0 + cw) * 128] = xb
                elb = np.full((cw * 128, 3), EL_NEG, np.float32)
                elb[m] = el1_n[srcs[m]]
                # layout [p, c*3+h]
                el1e[k][:, g0 * 3:(g0 + cw) * 3] = \
                    elb.reshape(cw, 128, 3).transpose(1, 0, 2).reshape(
                        128, cw * 3)
                g0 += cw

    # ---- per-core er1 (own dst nodes, q order); layout [p, t*3+h] ----
    er1e = np.zeros((NCORES, 128, TPC * 3), np.float32)
    for k in range(NCORES):
        tmp = np.zeros((NSL, 3), np.float32)
        sel = core == k
        tmp[q[sel]] = er1_n[sel]
        er1e[k] = tmp.reshape(TPC, 128, 3).transpose(1, 0, 2).reshape(
            128, TPC * 3)

    patch3 = np.zeros((1, ROWE), np.float32)
    patch3[:, F:F + 3] = EL_NEG

    Wpc = np.zeros((F, 2 * CLS), np.float32)
    Wpc[:, :CLS] = Wp[:F]
    Wpc[:, CLS:] = Wp[F:]

    consts = {
        "W1f": np.ascontiguousarray(W1aug[:, :F]).astype(bf16),
        "W2a": np.ascontiguousarray(W2aug[:96]).astype(bf16),
        "W2b": np.ascontiguousarray(W2aug[96:]).astype(bf16),
        "Wpa": np.ascontiguousarray(Wpc[:96]).astype(bf16),
        "Wpb": np.ascontiguousarray(Wpc[96:]).astype(bf16),
        "b1bc": np.tile(np.asarray(b1, np.float32), (128, 1)),
        "b2bc": np.tile(np.asarray(b2, np.float32), (128, 1)),
        "bpbc": np.tile(np.asarray(bp, np.float32), (128, 1)),
        "eyebf": np.eye(128, dtype=np.float32).astype(bf16),
        "patch3": patch3.astype(bf16),
    }
    in_maps = []
    for k in range(NCORES):
        m = dict(consts)
        m["XeT"] = XeT[k]
        m["el1e"] = el1e[k]
        m["er1e"] = er1e[k]
        m["idxA"] = idxw[0][k]
        m["idxM"] = idxw[1][k]
        m["idxB"] = idxw[2][k]
        in_maps.append(m)

    sig = tuple(tuple(int(x) for x in CW[w]) for w in range(3))
    return sig, in_maps, orig, sW, CW


# ======================================================================
# device program
# ======================================================================

def _build_program(sig):
    import sys
    if '/opt/trn_rl_repo' not in sys.path:
        sys.path.insert(0, '/opt/trn_rl_repo')
    import concourse.bass as bass
    import concourse.tile as tile
    from concourse import mybir
    from concourse import library_config

    CW = [list(x) for x in sig]
    sW = [[0] * TPC for _ in range(3)]
    for w in range(3):
        for t in range(1, TPC):
            sW[w][t] = sW[w][t - 1] + CW[w][t - 1]
    TC = [sW[w][-1] + CW[w][-1] for w in range(3)]
    NCH = [CW[0][t] + CW[1][t] + CW[2][t] for t in range(TPC)]
    sT = [0] * TPC
    for t in range(1, TPC):
        sT[t] = sT[t - 1] + NCH[t - 1]
    TOTCH = sT[-1] + NCH[-1]
    NCHMAX = max(NCH)

    fp32 = mybir.dt.float32
    bf16 = mybir.dt.bfloat16
    i16 = mybir.dt.int16
    AF = mybir.ActivationFunctionType
    OP = mybir.AluOpType
    AX = mybir.AxisListType

    nc = bass.Bass("TRN2", target_bir_lowering=False, debug=False,
                   num_devices=NCORES, num_swdge_queues=NQ)

    XeT = nc.declare_dram_parameter("XeT", [FIN, TOTCH * 128], bf16,
                                    isOutput=False)
    el1e_in = nc.declare_dram_parameter("el1e", [128, TOTCH * 3], fp32,
                                        isOutput=False)
    er1e_in = nc.declare_dram_parameter("er1e", [128, TPC * 3], fp32,
                                        isOutput=False)
    W1f = nc.declare_dram_parameter("W1f", [FIN, F], bf16, isOutput=False)
    W2a = nc.declare_dram_parameter("W2a", [96, ROWE], bf16, isOutput=False)
    W2b = nc.declare_dram_parameter("W2b", [96, ROWE], bf16, isOutput=False)
    Wpa = nc.declare_dram_parameter("Wpa", [96, 2 * CLS], bf16,
                                    isOutput=False)
    Wpb = nc.declare_dram_parameter("Wpb", [96, 2 * CLS], bf16,
                                    isOutput=False)
    b1bc = nc.declare_dram_parameter("b1bc", [128, F], fp32, isOutput=False)
    b2bc = nc.declare_dram_parameter("b2bc", [128, F], fp32, isOutput=False)
    bpbc = nc.declare_dram_parameter("bpbc", [128, CLS], fp32, isOutput=False)
    eye_in = nc.declare_dram_parameter("eyebf", [128, 128], bf16,
                                       isOutput=False)
    patch3 = nc.declare_dram_parameter("patch3", [1, ROWE], bf16,
                                       isOutput=False)
    idx_in = [nc.declare_dram_parameter(nm, [128, 8 * TC[w]], i16,
                                        isOutput=False)
              for w, nm in enumerate(("idxA", "idxM", "idxB"))]
    score_out = nc.declare_dram_parameter(
        "score_out", [TOTCH * 128, CLS], fp32, isOutput=True)

    with tile.TileContext(nc, num_cores=NCORES) as tc:
        with (
            tc.tile_pool(name="consts", bufs=1) as cpool,
            tc.tile_pool(name="dram", bufs=1, space="DRAM") as dpool,
            tc.tile_pool(name="xe", bufs=5) as xepool,
            tc.tile_pool(name="ge", bufs=2) as gepool,
            tc.tile_pool(name="brow", bufs=4) as brpool,
            tc.tile_pool(name="ga", bufs=6) as gApool,
            tc.tile_pool(name="gm", bufs=4) as gMpool,
            tc.tile_pool(name="gb", bufs=6) as gBpool,
            tc.tile_pool(name="lgp", bufs=2) as lgpool,
            tc.tile_pool(name="exp", bufs=3) as expool,
            tc.tile_pool(name="sm", bufs=4) as smpool,
            tc.tile_pool(name="hb", bufs=3) as hpool,
            tc.tile_pool(name="sc", bufs=2) as scpool,
            tc.tile_pool(name="pf1", bufs=3, space="PSUM") as f1pool,
            tc.tile_pool(name="pacc", bufs=1, space="PSUM") as apool,
            tc.tile_pool(name="ptp", bufs=1, space="PSUM") as tppool,
            tc.tile_pool(name="prow", bufs=2, space="PSUM") as rwpool,
            tc.tile_pool(name="per", bufs=1, space="PSUM") as erpool,
        ):
            # ---- DRAM internals ----
            f2sl = dpool.tile([NSL, ROWE], bf16, name="f2sl")
            T2 = dpool.tile([NTAB, ROWE], bf16, name="T2",
                            addr_space="Shared")
            hssl = dpool.tile([NSL, 128], bf16, name="hssl")
            HSD = dpool.tile([NTAB, 128], bf16, name="HSD",
                             addr_space="Shared")

            # ---- consts ----
            eye_sb = cpool.tile([128, 128], bf16, name="eye_sb")
            w1_sb = cpool.tile([FIN, F], bf16, name="w1_sb")
            w2a_sb = cpool.tile([96, ROWE], bf16, name="w2a_sb")
            w2b_sb = cpool.tile([96, ROWE], bf16, name="w2b_sb")
            wpa_sb = cpool.tile([96, 2 * CLS], bf16, name="wpa_sb")
            wpb_sb = cpool.tile([96, 2 * CLS], bf16, name="wpb_sb")
            b1_sb = cpool.tile([128, F], fp32, name="b1_sb")
            b2_sb = cpool.tile([128, F], fp32, name="b2_sb")
            bp_sb = cpool.tile([128, CLS], fp32, name="bp_sb")
            patch_sb = cpool.tile([1, ROWE], bf16, name="patch_sb")
            el1_sb = cpool.tile([128, TOTCH * 3], fp32, name="el1_sb")
            er1_sb = cpool.tile([128, TPC * 3], fp32, name="er1_sb")
            ix_sb = [cpool.tile([128, 8 * TC[w]], i16, name=f"ix{w}_sb")
                     for w in range(3)]
            er2_own = cpool.tile([128, TPC * 3], bf16, name="er2_own")
            hd_own = cpool.tile([128, TPC * CLS], bf16, name="hd_own")
            for sb, dr in [(eye_sb, eye_in), (w1_sb, W1f), (w2a_sb, W2a),
                           (w2b_sb, W2b), (wpa_sb, Wpa), (wpb_sb, Wpb),
                           (b1_sb, b1bc), (b2_sb, b2bc), (bp_sb, bpbc),
                           (patch_sb, patch3), (el1_sb, el1e_in),
                           (er1_sb, er1e_in), (ix_sb[0], idx_in[0]),
                           (ix_sb[1], idx_in[1]), (ix_sb[2], idx_in[2])]:
                nc.sync.dma_start(sb[:], dr[:])

            nc.gpsimd.load_library(library_config.mlp)

            _regs = {}
            _qctr = [0]

            def next_q():
                qn = _qctr[0] % NQ
                _qctr[0] += 1
                return qn

            def nidx_reg(v):
                if v not in _regs:
                    _regs[v] = nc.gpsimd.to_reg(v)
                return _regs[v]

            # ---- shared softmax prologue per tile ----
            def softmax_tile(t, lg_src_fn, lidx):
                nch = NCH[t]
                lg = lgpool.tile([128, NCHMAX, 3], fp32, name="lg", tag="lg")
                lg_src_fn(lg, nch)
                lr = smpool.tile([128, NCHMAX, 3], fp32, name="lr", tag="lr")
                nc.scalar.mul(lr[:, :nch, :], lg[:, :nch, :], NEG)
                nc.vector.tensor_tensor(out=lr[:, :nch, :],
                                        in0=lr[:, :nch, :],
                                        in1=lg[:, :nch, :], op=OP.max)
                ex = expool.tile([128, NCHMAX, 3], bf16, name="ex", tag="ex")
                nc.scalar.activation(ex[:, :nch, :], lr[:, :nch, :], AF.Exp)
                den = smpool.tile([128, 3], fp32, name="den", tag="den")
                nc.vector.tensor_reduce(
                    out=den[:],
                    in_=ex[:, :nch, :].rearrange("p c h -> p h c"),
                    axis=AX.X, op=OP.add)
                rden = smpool.tile([128, 3], fp32, name="rden", tag="rden")
                nc.vector.reciprocal(rden[:], den[:])
                return ex, rden

            # ---- h epilogue per tile (shared) ----
            def finish_tile(t, acc_ap, rden, lidx, out_fn):
                hf = hpool.tile([128, F], fp32, name="hf", tag="hf")
                for h in range(H):
                    nc.vector.tensor_scalar_mul(
                        hf[:, h * D:(h + 1) * D],
                        acc_ap[:, h * D:(h + 1) * D],
                        rden[:, h:h + 1])
                nc.vector.tensor_tensor(
                    out=hf[:], in0=hf[:],
                    in1=b1_sb[:] if lidx == 0 else b2_sb[:], op=OP.add)
                hbf = hpool.tile([128, F], bf16, name="hbf", tag="hbf")
                nc.scalar.activation(hbf[:], hf[:], AF.Relu)

                tpa = tppool.tile([96, 128], bf16, name="tpa", tag="tp")
                nc.tensor.transpose(out=tpa[:], in_=hbf[:, 0:96],
                                    identity=eye_sb[:])
                hta = smpool.tile([96, 128], bf16, name="hta", tag="hta")
                nc.vector.tensor_copy(hta[:], tpa[:])
                tpb = tppool.tile([96, 128], bf16, name="tpb", tag="tp")
                nc.tensor.transpose(out=tpb[:], in_=hbf[:, 96:F],
                                    identity=eye_sb[:])
                htb = smpool.tile([96, 128], bf16, name="htb", tag="htb")
                nc.vector.tensor_copy(htb[:], tpb[:])
                out_fn(t, hta, htb)

            # ---- skewed-stage pipeline driver ----
            def pipeline(stages, n=TPC):
                maxsk = max(sk for sk, _ in stages)
                for i in range(n + maxsk):
                    for sk, fn in stages:
                        t = i - sk
                        if 0 <= t < n:
                            fn(t)

            # ================= layer 1: Xe stream, no gathers ============
            xe_bufs = {}
            xnext = [0]

            def ensure_xe(upto):
                while xnext[0] * GXE < min(upto, TOTCH):
                    j = xnext[0]
                    lo = j * GXE
                    cnt = min(GXE, TOTCH - lo)
                    xt = xepool.tile([128, GXE * 128], bf16, name="xe",
                                     tag="xe")
                    nc.sync.dma_start(xt[:, :cnt * 128],
                                      XeT[:, lo * 128:(lo + cnt) * 128])
                    xe_bufs[j] = xt
                    xnext[0] += 1

            def xe_pieces(t):
                out = []
                lo, n = sT[t], NCH[t]
                while n > 0:
                    j = lo // GXE
                    off = lo - j * GXE
                    m = min(n, GXE - off)
                    out.append((j, off, m))
                    lo += m
                    n -= m
                return out

            def run_layer1(out_fn):
                st = {}

                def stA(t):
                    ensure_xe(sT[t] + NCH[t])

                def stB(t):
                    nch = NCH[t]

                    def lg_src(lg, nch=nch, t=t):
                        nc.vector.tensor_tensor(
                            out=lg[:, :nch, :],
                            in0=el1_sb[:, sT[t] * 3:(sT[t] + nch) * 3]
                            .rearrange("p (c h) -> p c h", h=3),
                            in1=er1_sb[:, t * 3:t * 3 + 3]
                            .unsqueeze(1).to_broadcast([128, nch, 3]),
                            op=OP.add)

                    st[t] = softmax_tile(t, lg_src, 0)

                def stC(t):
                    nch = NCH[t]
                    ex, _ = st[t]
                    ge = gepool.tile([128, NCHMAX, F], bf16, name="ge",
                                     tag="ge")
                    acc = apool.tile([128, F], fp32, name="acc", tag="acc")
                    st[(t, 'acc')] = acc
                    co = 0
                    for (j, off, m) in xe_pieces(t):
                        xt = xe_bufs[j]
                        c = 0
                        while c < m:
                            pp = min(2, m - c)
                            f1g = f1pool.tile([128, 2, ROWE], fp32,
                                              name="f1g", tag="f1g")
                            for jj in range(pp):
                                nc.tensor.matmul(
                                    out=f1g[:, jj, 0:F],
                                    lhsT=xt[:, (off + c + jj) * 128:
                                            (off + c + jj + 1) * 128],
                                    rhs=w1_sb[:], start=True, stop=True)
                            nc.vector.tensor_tensor(
                                out=ge[:, co + c:co + c + pp, :]
                                .rearrange("p c (h d) -> p c h d", h=H),
                                in0=f1g[:, :pp, 0:F]
                                .rearrange("p c (h d) -> p c h d", h=H),
                                in1=ex[:, co + c:co + c + pp, :]
                                .unsqueeze(3).to_broadcast([128, pp, H, D]),
                                op=OP.mult)
                            for jj in range(pp):
                                nc.tensor.matmul(
                                    out=acc[:], lhsT=eye_sb[:],
                                    rhs=ge[:, co + c + jj, :],
                                    start=(co + c + jj == 0),
                                    stop=(co + c + jj == nch - 1))
                            c += pp
                        co += m

                def stD(t):
                    ex, rden = st.pop(t)
                    acc = st.pop((t, 'acc'))
                    finish_tile(t, acc[:], rden, 0, out_fn)

                pipeline([(0, stA), (1, stB), (3, stD), (2, stC)])

            # ================= gather-based layer (layer 2) ==============

            gpools = (gApool, gMpool, gBpool)

            def run_layer_gather(T, lidx, out_fn):
                Tw = [T[W_OFF[w]:W_OFF[w] + WIN, :] for w in range(3)]
                nextc = [0, 0, 0]
                bufs = [{}, {}, {}]

                def ensure(w, upto_chunk):
                    while nextc[w] * GCH < min(upto_chunk, TC[w]):
                        j = nextc[w]
                        lo = j * GCH
                        cnt = min(GCH, TC[w] - lo)
                        gt = gpools[w].tile([128, GCH, ROWE], bf16,
                                            name=f"g{w}", tag=f"g{w}")
                        nc.gpsimd.dma_gather(
                            gt[:, :cnt, :], Tw[w],
                            ix_sb[w][:, 8 * lo:8 * (lo + cnt)],
                            128 * cnt, nidx_reg(128 * cnt), ROWE,
                            single_packet=False, queue_num=next_q())
                        bufs[w][j] = gt
                        nextc[w] += 1

                def pieces(w, t):
                    out = []
                    lo, n = sW[w][t], CW[w][t]
                    while n > 0:
                        j = lo // GCH
                        off = lo - j * GCH
                        m = min(n, GCH - off)
                        out.append((j, off, m))
                        lo += m
                        n -= m
                    return out

                st = {}

                # emit ALL gather calls upfront, ordered by first-consumer
                # tile; pool buffer reuse provides the backpressure that
                # keeps ~bufs calls in flight across the 4 SWDGE queues.
                order = []
                for w in range(3):
                    ncall = (TC[w] + GCH - 1) // GCH
                    for j in range(ncall):
                        fct = next(t for t in range(TPC)
                                   if sW[w][t] + CW[w][t] > j * GCH)
                        order.append((fct, w, j))
                order.sort()
                for (_, w, j) in order:
                    ensure(w, j * GCH + 1)

                def stB(t):
                    nch = NCH[t]
                    pcs = [(w, *pc) for w in range(3)
                           for pc in pieces(w, t)]

                    def lg_src(lg, nch=nch, t=t, pcs=pcs):
                        co = 0
                        for (w, j, off, m) in pcs:
                            gt = bufs[w][j]
                            nc.vector.tensor_tensor(
                                out=lg[:, co:co + m, :],
                                in0=gt[:, off:off + m, F:F + 3],
                                in1=er2_own[:, t * 3:t * 3 + 3]
                                .unsqueeze(1).to_broadcast([128, m, 3]),
                                op=OP.add)
                            co += m

                    st[t] = softmax_tile(t, lg_src, lidx)

                def stC(t):
                    nch = NCH[t]
                    ex, _ = st[t]
                    pcs = [(w, *pc) for w in range(3)
                           for pc in pieces(w, t)]
                    acc = apool.tile([128, F], fp32, name="acc", tag="acc")
                    st[(t, 'acc')] = acc
                    co = 0
                    first = True
                    for (w, j, off, m) in pcs:
                        gt = bufs[w][j]
                        fv = gt[:, off:off + m, 0:F].rearrange(
                            "p c (h d) -> p c h d", h=H)
                        nc.vector.tensor_tensor(
                            out=fv, in0=fv,
                            in1=ex[:, co:co + m, :]
                            .unsqueeze(3).to_broadcast([128, m, H, D]),
                            op=OP.mult)
                        for c in range(m):
                            nc.tensor.matmul(
                                out=acc[:], lhsT=eye_sb[:],
                                rhs=gt[:, off + c, 0:F],
                                start=first,
                                stop=(co + c == nch - 1))
                            first = False
                        co += m

                def stD(t):
                    ex, rden = st.pop(t)
                    acc = st.pop((t, 'acc'))
                    finish_tile(t, acc[:], rden, lidx, out_fn)

                pipeline([(0, stB), (2, stD), (1, stC)])

            # ---- layer 1 output: build T2 rows, AllGather at end ----
            def l1_out(t, hta, htb):
                f2p = rwpool.tile([128, ROWE], fp32, name="f2p", tag="prow")
                nc.tensor.matmul(out=f2p[:], lhsT=hta[:], rhs=w2a_sb[:],
                                 start=True, stop=False)
                nc.tensor.matmul(out=f2p[:], lhsT=htb[:], rhs=w2b_sb[:],
                                 start=False, stop=True)
                f2s = brpool.tile([128, ROWE], bf16, name="f2s", tag="f1s")
                nc.vector.tensor_copy(f2s[:], f2p[:])
                nc.scalar.activation(er2_own[:, t * 3:t * 3 + 3],
                                     f2p[:, F + 3:F + 6], AF.Copy)
                nc.sync.dma_start(f2sl[t * 128:(t + 1) * 128, :], f2s[:])
                if t == TPC - 1:
                    nc.sync.dma_start(f2sl[6250:6251, :], patch_sb[0:1, :])
                    nc.gpsimd.collective_compute(
                        "AllGather", mybir.AluOpType.bypass,
                        replica_groups=[list(range(NCORES))],
                        ins=[f2sl[:]], outs=[T2[:]])

            run_layer1(l1_out)

            # ---- layer 2 output: hs rows + hd, AllGather at end ----
            def l2_out(t, hta, htb):
                hsp = erpool.tile([128, 2 * CLS], fp32, name="hsp", tag="er")
                nc.tensor.matmul(out=hsp[:], lhsT=hta[:], rhs=wpa_sb[:],
                                 start=True, stop=False)
                nc.tensor.matmul(out=hsp[:], lhsT=htb[:], rhs=wpb_sb[:],
                                 start=False, stop=True)
                hss = brpool.tile([128, 128], bf16, name="hss", tag="f1s")
                nc.scalar.activation(hss[:, 0:CLS], hsp[:, 0:CLS], AF.Copy)
                nc.vector.tensor_tensor(
                    out=hd_own[:, t * CLS:(t + 1) * CLS],
                    in0=hsp[:, CLS:2 * CLS], in1=bp_sb[:], op=OP.add)
                nc.sync.dma_start(hssl[t * 128:(t + 1) * 128, :], hss[:])
                if t == TPC - 1:
                    nc.gpsimd.collective_compute(
                        "AllGather", mybir.AluOpType.bypass,
                        replica_groups=[list(range(NCORES))],
                        ins=[hssl[:]], outs=[HSD[:]])

            run_layer_gather(T2, 1, l2_out)

            # ---- score pass ----
            def run_score():
                Hw = [HSD[W_OFF[w]:W_OFF[w] + WIN, :] for w in range(3)]
                nextc = [0, 0, 0]
                bufs = [{}, {}, {}]

                def ensure_s(w, upto):
                    while nextc[w] * GCH < min(upto, TC[w]):
                        j = nextc[w]
                        lo = j * GCH
                        cnt = min(GCH, TC[w] - lo)
                        # reuse the layer-2 gather buffers (same tag), but
                        # viewed as [128, 2*GCH, 128] for 256B score rows
                        gt0 = gpools[w].tile([128, GCH, ROWE], bf16,
                                             name=f"g{w}", tag=f"g{w}")
                        gt = gt0[:].rearrange("p c (a e) -> p (c a) e", a=2)
                        nc.gpsimd.dma_gather(
                            gt[:, :cnt, :], Hw[w],
                            ix_sb[w][:, 8 * lo:8 * (lo + cnt)],
                            128 * cnt, nidx_reg(128 * cnt), 128,
                            single_packet=False, queue_num=next_q())
                        bufs[w][j] = gt
                        nextc[w] += 1

                def pieces_s(t):
                    out = []
                    for w in range(3):
                        lo, n = sW[w][t], CW[w][t]
                        while n > 0:
                            j = lo // GCH
                            off = lo - j * GCH
                            m = min(n, GCH - off)
                            out.append((w, j, off, m))
                            lo += m
                            n -= m
                    return out

                order = []
                for w in range(3):
                    ncall = (TC[w] + GCH - 1) // GCH
                    for j in range(ncall):
                        fct = next(t for t in range(TPC)
                                   if sW[w][t] + CW[w][t] > j * GCH)
                        order.append((fct, w, j))
                order.sort()
                for (_, w, j) in order:
                    ensure_s(w, j * GCH + 1)

                def stB(t):
                    nch = NCH[t]
                    sc = scpool.tile([128, NCHMAX, CLS], fp32, name="sc",
                                     tag="sc")
                    co = 0
                    for (w, j, off, m) in pieces_s(t):
                        gt = bufs[w][j]
                        nc.vector.tensor_tensor(
                            out=sc[:, co:co + m, :],
                            in0=gt[:, off:off + m, 0:CLS],
                            in1=hd_own[:, t * CLS:(t + 1) * CLS]
                            .unsqueeze(1).to_broadcast([128, m, CLS]),
                            op=OP.add)
                        co += m
                    out_v = score_out[sT[t] * 128:(sT[t] + nch) * 128, :] \
                        .rearrange("(p c) j -> p c j", p=128)
                    nc.sync.dma_start(out_v, sc[:, :nch, :])

                pipeline([(0, stB)])

            run_score()

    mybir.codegen_inst_isa_subclasses(nc)
    _cap_waits(nc, mybir)
    return nc


def _cap_waits(nc, mybir, lim=1):
    """Walrus embeds at most `lim` semaphore waits per HW instruction.
    Move excess waits onto same-engine NoOps inserted just before."""
    eng_map = {
        mybir.EngineType.PE: nc.tensor,
        mybir.EngineType.DVE: nc.vector,
        mybir.EngineType.Activation: nc.scalar,
        mybir.EngineType.Pool: nc.gpsimd,
        mybir.EngineType.SP: nc.sync,
    }
    scratch = nc.main_func.blocks[-1].instructions
    for bb in nc.main_func.blocks:
        out = []
        for ins in bb.instructions:
            si = ins.sync_info
            waits = list(si.on_wait) if si is not None and si.on_wait else []
            if len(waits) > lim:
                keep = waits[-lim:]
                excess = waits[:-lim]
                eng = eng_map.get(ins.engine)
                assert eng is not None, f"no engine for {ins}"
                while excess:
                    grp, excess = excess[:lim], excess[lim:]
                    eng.nop(hint="waitsplit", nofuse=True)
                    nop = scratch.pop()
                    nop.sync_info = mybir.SyncInfo(on_wait=grp, on_update=[])
                    out.append(nop)
                ins.sync_info = mybir.SyncInfo(
                    on_wait=keep, on_update=list(si.on_update or []))
            out.append(ins)
        bb.instructions[:] = out


# ======================================================================
# entry point
# ======================================================================

def kernel(src, dst, nfeats, efeats, W1, al1, ar1, b1, W2, al2, ar2, b2,
           Wp, bp, _collect=None):
    import sys
    if '/opt/trn_rl_repo' not in sys.path:
        sys.path.insert(0, '/opt/trn_rl_repo')
    from concourse.bass_utils import run_bass_kernel_spmd

    sig, in_maps, orig, sW_np, CW_np = _prep(
        src, dst, nfeats, W1, al1, ar1, b1, W2, al2, ar2, b2, Wp, bp)
    if sig not in _COMPILED:
        _COMPILED[sig] = _build_program(sig)
    nc = _COMPILED[sig]

    kw = dict(_collect or {})
    kw.pop("results", None)
    res = run_bass_kernel_spmd(nc, in_maps, list(range(NCORES)), **kw)
    if _collect is not None:
        _collect["results"] = res

    # assemble: device slot order is tile-major, then window A,M,B chunks
    NCHt = (CW_np[0] + CW_np[1] + CW_np[2])
    sT = np.zeros(TPC, np.int64)
    sT[1:] = np.cumsum(NCHt[:-1])
    out = np.zeros((E, CLS), np.float32)
    for k in range(NCORES):
        sc = np.asarray(res.results[k]["score_out"])
        for t in range(TPC):
            nch = int(NCHt[t])
            # device rows for tile t: sT[t]*128 + p*nch + c
            blk = sc[sT[t] * 128:(sT[t] + nch) * 128].reshape(128, nch, CLS)
            co = 0
            for w in range(3):
                lo, n = int(sW_np[w][t]), int(CW_np[w][t])
                # orig index layout: position (lo+c)*128 + p
                o = orig[w][k][lo * 128:(lo + n) * 128].reshape(n, 128)
                rows = blk[:, co:co + n].transpose(1, 0, 2)  # [n,128,CLS]
                m = o >= 0
                out[o[m]] = rows[m]
                co += n
    return out


# revision 26
# speedup vs baseline: 1.1446x; 1.1446x over previous
"""GAT (2-layer) + edge MLP predictor on 8 TRN2 NeuronCores — v3.

Design (edge/1D graph partition parallelism, dst-aligned column-major):
  - dst nodes ranked by in-degree (desc), dealt round-robin to cores;
    bands of 1024 ranks form one tile across all 8 cores, so per-tile
    max degree is uniform and shared (SPMD single program).
  - Edge slots are COLUMN-MAJOR: slot (p, c) of a tile is the c-th edge
    of dst node p.  dst == partition, so softmax denominators are
    free-axis reductions.
  - LAYER 1 does not gather at all: the host pre-permutes the INPUT
    node features into edge-slot order (XeT stream) plus per-slot
    attention logits el1 and per-node er1, so layer-1 edge rows are
    produced by plain matmuls against W1 on a linear DMA stream.
  - LAYER 2 + SCORE gather per-edge rows from device-built tables with
    InstDMAGatherAnt.  Desc-gen is the bottleneck (one Q7 core-pair per
    SWDGE queue), so calls round-robin across 4 queues and are issued
    with deep lookahead so 4 desc-gens run concurrently.
  - The 50176-row tables exceed int16, so three overlapping 32768-row
    windows (A/M/B) are used; per-node edges are split between windows
    by a small per-tile LP.  Padding slots gather a special row with
    el = -200 so exp() zeroes them (no masks).
  - Layer-2 table and score table are assembled via chunked AllGather
    interleaved with compute.
  - Score: score_e = hs[src_e] (gathered) + hd[dst_e] (local).
"""

import math

import numpy as np

# ---- problem constants ----
N = 50000
E = 800000
FIN = 128
H = 3
D = 64
F = 192
CLS = 10
NEG = 0.2
NCORES = 8
NPC = 6250
TPC = 49
ROWE = 256          # table row elems (bf16)
NSL = 6272          # padded nodes per core (49 tiles)
NTAB = NCORES * NSL                     # 50176 = 392*128
WIN = 32768
W_OFF = (0, 8704, NTAB - WIN)           # A, M, B window bases
SPEC = 3 * NSL + 6250                   # special row: core3 pad slot,
                                        # inside the A&M&B overlap
PAD_IDX = (SPEC, SPEC - W_OFF[1], SPEC - W_OFF[2])
EL_NEG = -200.0
GCH = 10            # gather chunks per call (1280 idx)
GXE = 16            # xe stream chunks per DMA call
LA_CH = 24          # gather lookahead chunks per window
NQ = 4              # SWDGE queues

_COMPILED = {}


# ======================================================================
# host-side preprocessing
# ======================================================================

def _prep(src, dst, nfeats, W1, al1, ar1, b1, W2, al2, ar2, b2, Wp, bp):
    import ml_dtypes
    bf16 = ml_dtypes.bfloat16
    src = np.asarray(src).astype(np.int64)
    dst = np.asarray(dst).astype(np.int64)

    deg = np.bincount(dst, minlength=N)
    assert deg.min() >= 1, "0-degree dst nodes unsupported"
    order = np.argsort(-deg, kind='stable')
    rank = np.empty(N, np.int64)
    rank[order] = np.arange(N)
    core = rank % NCORES
    q = rank // NCORES
    tpos = core * NSL + q

    e_core = core[dst]
    e_tile = (q // 128)[dst]
    e_row = (q % 128)[dst]
    e_tpos = tpos[src]

    # segments: 0 A-only, 1 AM, 2 AMB, 3 MB, 4 B-only
    WM, WB = W_OFF[1], W_OFF[2]
    seg = np.zeros(E, np.int8)
    seg[(e_tpos >= WM) & (e_tpos < WB)] = 1
    seg[(e_tpos >= WB) & (e_tpos < WIN)] = 2
    seg[(e_tpos >= WIN) & (e_tpos < WM + WIN)] = 3
    seg[e_tpos >= WM + WIN] = 4

    key = (e_core * TPC + e_tile) * 128 + e_row
    nk = NCORES * TPC * 128
    cnt = np.zeros((5, nk), np.int64)
    for s in range(5):
        cnt[s] = np.bincount(key[seg == s], minlength=nk)
    cnt = cnt.reshape(5, NCORES, TPC, 128)

    CW = np.zeros((3, TPC), np.int64)          # caps per window per tile
    for t in range(TPC):
        aa = cnt[0, :, t, :].ravel()
        am = cnt[1, :, t, :].ravel()
        amb = cnt[2, :, t, :].ravel()
        mb = cnt[3, :, t, :].ravel()
        bb = cnt[4, :, t, :].ravel()
        d = aa + am + amb + mb + bb
        c1 = int(aa.max()); c2 = int(bb.max())
        c3 = int((aa + am).max()); c4 = int((bb + mb).max())
        c5 = int(d.max()); c6 = int((aa + bb).max())
        best = None
        for ca in range(c1, c5 + 2):
            for cm in range(max(0, c3 - ca), c5 + 2 - ca):
                cb = max(c2, c4 - cm, c5 - ca - cm, c6 - ca, 0)
                cost = ca + cm + cb
                if best is None or cost < best[0]:
                    best = (cost, ca, cm, cb)
        CW[0, t], CW[1, t], CW[2, t] = best[1:]

    sW = np.zeros((3, TPC), np.int64)          # chunk start per window
    sW[:, 1:] = np.cumsum(CW[:, :-1], axis=1)
    TC = CW.sum(axis=1)                        # total chunks per window

    idx16 = [np.full((NCORES, int(TC[w]) * 128), PAD_IDX[w], np.int64)
             for w in range(3)]
    orig = [np.full((NCORES, int(TC[w]) * 128), -1, np.int64)
            for w in range(3)]

    sort = np.argsort(key, kind='stable')
    bnd = np.searchsorted(key[sort], np.arange(nk + 1))
    for k in range(NCORES):
        for t in range(TPC):
            ca, cm, cb = CW[0, t], CW[1, t], CW[2, t]
            for p in range(128):
                b0 = bnd[(k * TPC + t) * 128 + p]
                b1_ = bnd[(k * TPC + t) * 128 + p + 1]
                eids = sort[b0:b1_]
                sg = seg[eids]
                l0 = list(eids[sg == 0]); l1 = list(eids[sg == 1])
                l2 = list(eids[sg == 2]); l3 = list(eids[sg == 3])
                l4 = list(eids[sg == 4])
                la = l0
                take = min(ca - len(la), len(l1))
                la = la + l1[:take]; rem1 = l1[take:]
                take = min(ca - len(la), len(l2))
                la = la + l2[:take]; rem2 = l2[take:]
                lm = rem1
                take = min(cm - len(lm), len(rem2))
                lm = lm + rem2[:take]; rem2 = rem2[take:]
                take = min(cm - len(lm), len(l3))
                lm = lm + l3[:take]
                lb = l3[take:] + l4 + rem2
                assert len(la) <= ca and len(lm) <= cm and len(lb) <= cb
                for w, lst in ((0, la), (1, lm), (2, lb)):
                    for c, e in enumerate(lst):
                        pos = (sW[w, t] + c) * 128 + p
                        idx16[w][k, pos] = e_tpos[e] - W_OFF[w]
                        orig[w][k, pos] = e

    def wrap16(a):
        n = a.shape[1]
        w = a.reshape(NCORES, n // 16, 16).transpose(0, 2, 1)
        return np.ascontiguousarray(np.tile(w, (1, 8, 1))).astype(np.int16)

    idxw = [wrap16(x) for x in idx16]

    # ---- weights ----
    def aug_w(W, al, ar):
        fin = W.shape[0]
        Wg = np.zeros((fin, ROWE), np.float32)
        Wg[:, :F] = W
        for h in range(H):
            Wg[:, F + h] = W[:, h * D:(h + 1) * D] @ al[h]
            Wg[:, F + 3 + h] = W[:, h * D:(h + 1) * D] @ ar[h]
        return Wg

    W1aug = aug_w(np.asarray(W1, np.float32), al1, ar1)
    W2aug = aug_w(np.asarray(W2, np.float32), al2, ar2)

    X32 = np.asarray(nfeats, np.float32)
    el1_n = X32 @ W1aug[:, F:F + 3]            # [N, 3]
    er1_n = X32 @ W1aug[:, F + 3:F + 6]        # [N, 3]
    XT_bf = np.ascontiguousarray(X32.T).astype(bf16)   # [FIN, N]

    # ---- per-tile chunk bookkeeping (host copies of device maps) ----
    NCHt = CW.sum(axis=0)
    sT = np.zeros(TPC, np.int64)
    sT[1:] = np.cumsum(NCHt[:-1])
    TOTCH = int(NCHt.sum())

    # ---- layer-1 edge-order streams ----
    XeT = np.zeros((NCORES, FIN, TOTCH * 128), bf16)
    el1e = np.full((NCORES, 128, TOTCH * 3), EL_NEG, np.float32)
    for k in range(NCORES):
        for t in range(TPC):
            g0 = sT[t]
            for w in range(3):
                cw = int(CW[w, t])
                if cw == 0:
                    continue
                lo = int(sW[w, t])
                blk = orig[w][k][lo * 128:(lo + cw) * 128]   # (c,p) order
                m = blk >= 0
                srcs = np.zeros(cw * 128, np.int64)
                srcs[m] = src[blk[m]]
                xb = XT_bf[:, srcs].copy()
                xb[:, ~m] = 0
                XeT[k][:, g0 * 128:(g0 + cw) * 128] = xb
                elb = np.full((cw * 128, 3), EL_NEG, np.float32)
                elb[m] = el1_n[srcs[m]]
                # layout [p, c*3+h]
                el1e[k][:, g0 * 3:(g0 + cw) * 3] = \
                    elb.reshape(cw, 128, 3).transpose(1, 0, 2).reshape(
                        128, cw * 3)
                g0 += cw

    # ---- per-core er1 (own dst nodes, q order); layout [p, t*3+h] ----
    er1e = np.zeros((NCORES, 128, TPC * 3), np.float32)
    for k in range(NCORES):
        tmp = np.zeros((NSL, 3), np.float32)
        sel = core == k
        tmp[q[sel]] = er1_n[sel]
        er1e[k] = tmp.reshape(TPC, 128, 3).transpose(1, 0, 2).reshape(
            128, TPC * 3)

    patch3 = np.zeros((1, ROWE), np.float32)
    patch3[:, F:F + 3] = EL_NEG

    Wpc = np.zeros((F, 2 * CLS), np.float32)
    Wpc[:, :CLS] = Wp[:F]
    Wpc[:, CLS:] = Wp[F:]

    consts = {
        "W1f": np.ascontiguousarray(W1aug[:, :F]).astype(bf16),
        "W2a": np.ascontiguousarray(W2aug[:96]).astype(bf16),
        "W2b": np.ascontiguousarray(W2aug[96:]).astype(bf16),
        "Wpa": np.ascontiguousarray(Wpc[:96]).astype(bf16),
        "Wpb": np.ascontiguousarray(Wpc[96:]).astype(bf16),
        "b1bc": np.tile(np.asarray(b1, np.float32), (128, 1)),
        "b2bc": np.tile(np.asarray(b2, np.float32), (128, 1)),
        "bpbc": np.tile(np.asarray(bp, np.float32), (128, 1)),
        "eyebf": np.eye(128, dtype=np.float32).astype(bf16),
        "patch3": patch3.astype(bf16),
    }
    in_maps = []
    for k in range(NCORES):
        m = dict(consts)
        m["XeT"] = XeT[k]
        m["el1e"] = el1e[k]
        m["er1e"] = er1e[k]
        m["idxA"] = idxw[0][k]
        m["idxM"] = idxw[1][k]
        m["idxB"] = idxw[2][k]
        in_maps.append(m)

    sig = tuple(tuple(int(x) for x in CW[w]) for w in range(3))
    return sig, in_maps, orig, sW, CW


# ======================================================================
# device program
# ======================================================================

def _build_program(sig):
    import sys
    if '/opt/trn_rl_repo' not in sys.path:
        sys.path.insert(0, '/opt/trn_rl_repo')
    import concourse.bass as bass
    import concourse.tile as tile
    from concourse import mybir
    from concourse import library_config

    CW = [list(x) for x in sig]
    sW = [[0] * TPC for _ in range(3)]
    for w in range(3):
        for t in range(1, TPC):
            sW[w][t] = sW[w][t - 1] + CW[w][t - 1]
    TC = [sW[w][-1] + CW[w][-1] for w in range(3)]
    NCH = [CW[0][t] + CW[1][t] + CW[2][t] for t in range(TPC)]
    sT = [0] * TPC
    for t in range(1, TPC):
        sT[t] = sT[t - 1] + NCH[t - 1]
    TOTCH = sT[-1] + NCH[-1]
    NCHMAX = max(NCH)

    fp32 = mybir.dt.float32
    bf16 = mybir.dt.bfloat16
    i16 = mybir.dt.int16
    AF = mybir.ActivationFunctionType
    OP = mybir.AluOpType
    AX = mybir.AxisListType

    nc = bass.Bass("TRN2", target_bir_lowering=False, debug=False,
                   num_devices=NCORES, num_swdge_queues=NQ)

    XeT = nc.declare_dram_parameter("XeT", [FIN, TOTCH * 128], bf16,
                                    isOutput=False)
    el1e_in = nc.declare_dram_parameter("el1e", [128, TOTCH * 3], fp32,
                                        isOutput=False)
    er1e_in = nc.declare_dram_parameter("er1e", [128, TPC * 3], fp32,
                                        isOutput=False)
    W1f = nc.declare_dram_parameter("W1f", [FIN, F], bf16, isOutput=False)
    W2a = nc.declare_dram_parameter("W2a", [96, ROWE], bf16, isOutput=False)
    W2b = nc.declare_dram_parameter("W2b", [96, ROWE], bf16, isOutput=False)
    Wpa = nc.declare_dram_parameter("Wpa", [96, 2 * CLS], bf16,
                                    isOutput=False)
    Wpb = nc.declare_dram_parameter("Wpb", [96, 2 * CLS], bf16,
                                    isOutput=False)
    b1bc = nc.declare_dram_parameter("b1bc", [128, F], fp32, isOutput=False)
    b2bc = nc.declare_dram_parameter("b2bc", [128, F], fp32, isOutput=False)
    bpbc = nc.declare_dram_parameter("bpbc", [128, CLS], fp32, isOutput=False)
    eye_in = nc.declare_dram_parameter("eyebf", [128, 128], bf16,
                                       isOutput=False)
    patch3 = nc.declare_dram_parameter("patch3", [1, ROWE], bf16,
                                       isOutput=False)
    idx_in = [nc.declare_dram_parameter(nm, [128, 8 * TC[w]], i16,
                                        isOutput=False)
              for w, nm in enumerate(("idxA", "idxM", "idxB"))]
    score_out = nc.declare_dram_parameter(
        "score_out", [TOTCH * 128, CLS], fp32, isOutput=True)

    with tile.TileContext(nc, num_cores=NCORES) as tc:
        with (
            tc.tile_pool(name="consts", bufs=1) as cpool,
            tc.tile_pool(name="dram", bufs=1, space="DRAM") as dpool,
            tc.tile_pool(name="xe", bufs=4) as xepool,
            tc.tile_pool(name="ge", bufs=2) as gepool,
            tc.tile_pool(name="brow", bufs=4) as brpool,
            tc.tile_pool(name="ga", bufs=8) as gApool,
            tc.tile_pool(name="gm", bufs=5) as gMpool,
            tc.tile_pool(name="gb", bufs=8) as gBpool,
            tc.tile_pool(name="lgp", bufs=2) as lgpool,
            tc.tile_pool(name="exp", bufs=3) as expool,
            tc.tile_pool(name="sm", bufs=4) as smpool,
            tc.tile_pool(name="hb", bufs=3) as hpool,
            tc.tile_pool(name="sc", bufs=4) as scpool,
            tc.tile_pool(name="pf1", bufs=3, space="PSUM") as f1pool,
            tc.tile_pool(name="pacc", bufs=1, space="PSUM") as apool,
            tc.tile_pool(name="ptp", bufs=1, space="PSUM") as tppool,
            tc.tile_pool(name="prow", bufs=2, space="PSUM") as rwpool,
            tc.tile_pool(name="per", bufs=1, space="PSUM") as erpool,
        ):
            # ---- DRAM internals ----
            f2sl = dpool.tile([NSL, ROWE], bf16, name="f2sl")
            T2 = dpool.tile([NTAB, ROWE], bf16, name="T2",
                            addr_space="Shared")
            hssl = dpool.tile([NSL, 128], bf16, name="hssl")
            HSD = dpool.tile([NTAB, 128], bf16, name="HSD",
                             addr_space="Shared")

            # ---- consts ----
            eye_sb = cpool.tile([128, 128], bf16, name="eye_sb")
            w1_sb = cpool.tile([FIN, F], bf16, name="w1_sb")
            w2a_sb = cpool.tile([96, ROWE], bf16, name="w2a_sb")
            w2b_sb = cpool.tile([96, ROWE], bf16, name="w2b_sb")
            wpa_sb = cpool.tile([96, 2 * CLS], bf16, name="wpa_sb")
            wpb_sb = cpool.tile([96, 2 * CLS], bf16, name="wpb_sb")
            b1_sb = cpool.tile([128, F], fp32, name="b1_sb")
            b2_sb = cpool.tile([128, F], fp32, name="b2_sb")
            bp_sb = cpool.tile([128, CLS], fp32, name="bp_sb")
            patch_sb = cpool.tile([1, ROWE], bf16, name="patch_sb")
            el1_sb = cpool.tile([128, TOTCH * 3], fp32, name="el1_sb")
            er1_sb = cpool.tile([128, TPC * 3], fp32, name="er1_sb")
            ix_sb = [cpool.tile([128, 8 * TC[w]], i16, name=f"ix{w}_sb")
                     for w in range(3)]
            er2_own = cpool.tile([128, TPC * 3], bf16, name="er2_own")
            hd_own = cpool.tile([128, TPC * CLS], bf16, name="hd_own")
            for sb, dr in [(eye_sb, eye_in), (w1_sb, W1f), (w2a_sb, W2a),
                           (w2b_sb, W2b), (wpa_sb, Wpa), (wpb_sb, Wpb),
                           (b1_sb, b1bc), (b2_sb, b2bc), (bp_sb, bpbc),
                           (patch_sb, patch3), (el1_sb, el1e_in),
                           (er1_sb, er1e_in), (ix_sb[0], idx_in[0]),
                           (ix_sb[1], idx_in[1]), (ix_sb[2], idx_in[2])]:
                nc.sync.dma_start(sb[:], dr[:])

            nc.gpsimd.load_library(library_config.mlp)

            _regs = {}
            _qctr = [0]

            def next_q():
                qn = _qctr[0] % NQ
                _qctr[0] += 1
                return qn

            def nidx_reg(v):
                if v not in _regs:
                    _regs[v] = nc.gpsimd.to_reg(v)
                return _regs[v]

            # ---- shared softmax prologue per tile ----
            def softmax_tile(t, lg_src_fn, lidx):
                nch = NCH[t]
                lg = lgpool.tile([128, NCHMAX, 3], fp32, name="lg", tag="lg")
                lg_src_fn(lg, nch)
                lr = smpool.tile([128, NCHMAX, 3], fp32, name="lr", tag="lr")
                nc.scalar.mul(lr[:, :nch, :], lg[:, :nch, :], NEG)
                nc.vector.tensor_tensor(out=lr[:, :nch, :],
                                        in0=lr[:, :nch, :],
                                        in1=lg[:, :nch, :], op=OP.max)
                ex = expool.tile([128, NCHMAX, 3], bf16, name="ex", tag="ex")
                nc.scalar.activation(ex[:, :nch, :], lr[:, :nch, :], AF.Exp)
                den = smpool.tile([128, 3], fp32, name="den", tag="den")
                nc.vector.tensor_reduce(
                    out=den[:],
                    in_=ex[:, :nch, :].rearrange("p c h -> p h c"),
                    axis=AX.X, op=OP.add)
                rden = smpool.tile([128, 3], fp32, name="rden", tag="rden")
                nc.vector.reciprocal(rden[:], den[:])
                return ex, rden

            # ---- h epilogue per tile (shared) ----
            def finish_tile(t, acc_ap, rden, lidx, out_fn):
                hf = hpool.tile([128, F], fp32, name="hf", tag="hf")
                for h in range(H):
                    nc.vector.tensor_scalar_mul(
                        hf[:, h * D:(h + 1) * D],
                        acc_ap[:, h * D:(h + 1) * D],
                        rden[:, h:h + 1])
                nc.vector.tensor_tensor(
                    out=hf[:], in0=hf[:],
                    in1=b1_sb[:] if lidx == 0 else b2_sb[:], op=OP.add)
                hbf = hpool.tile([128, F], bf16, name="hbf", tag="hbf")
                nc.scalar.activation(hbf[:], hf[:], AF.Relu)

                tpa = tppool.tile([96, 128], bf16, name="tpa", tag="tp")
                nc.tensor.transpose(out=tpa[:], in_=hbf[:, 0:96],
                                    identity=eye_sb[:])
                hta = smpool.tile([96, 128], bf16, name="hta", tag="hta")
                nc.vector.tensor_copy(hta[:], tpa[:])
                tpb = tppool.tile([96, 128], bf16, name="tpb", tag="tp")
                nc.tensor.transpose(out=tpb[:], in_=hbf[:, 96:F],
                                    identity=eye_sb[:])
                htb = smpool.tile([96, 128], bf16, name="htb", tag="htb")
                nc.vector.tensor_copy(htb[:], tpb[:])
                out_fn(t, hta, htb)

            # ---- skewed-stage pipeline driver ----
            def pipeline(stages, n=TPC):
                maxsk = max(sk for sk, _ in stages)
                for i in range(n + maxsk):
                    for sk, fn in stages:
                        t = i - sk
                        if 0 <= t < n:
                            fn(t)

            # ================= layer 1: Xe stream, no gathers ============
            xe_bufs = {}
            xnext = [0]

            def ensure_xe(upto):
                while xnext[0] * GXE < min(upto, TOTCH):
                    j = xnext[0]
                    lo = j * GXE
                    cnt = min(GXE, TOTCH - lo)
                    xt = xepool.tile([128, GXE * 128], bf16, name="xe",
                                     tag="xe")
                    nc.sync.dma_start(xt[:, :cnt * 128],
                                      XeT[:, lo * 128:(lo + cnt) * 128])
                    xe_bufs[j] = xt
                    xnext[0] += 1

            def xe_pieces(t):
                out = []
                lo, n = sT[t], NCH[t]
                while n > 0:
                    j = lo // GXE
                    off = lo - j * GXE
                    m = min(n, GXE - off)
                    out.append((j, off, m))
                    lo += m
                    n -= m
                return out

            def run_layer1(out_fn):
                st = {}

                def stA(t):
                    ensure_xe(sT[t] + NCH[t])

                def stB(t):
                    nch = NCH[t]

                    def lg_src(lg, nch=nch, t=t):
                        nc.vector.tensor_tensor(
                            out=lg[:, :nch, :],
                            in0=el1_sb[:, sT[t] * 3:(sT[t] + nch) * 3]
                            .rearrange("p (c h) -> p c h", h=3),
                            in1=er1_sb[:, t * 3:t * 3 + 3]
                            .unsqueeze(1).to_broadcast([128, nch, 3]),
                            op=OP.add)

                    st[t] = softmax_tile(t, lg_src, 0)

                def stC(t):
                    nch = NCH[t]
                    ex, _ = st[t]
                    ge = gepool.tile([128, NCHMAX, F], bf16, name="ge",
                                     tag="ge")
                    acc = apool.tile([128, F], fp32, name="acc", tag="acc")
                    st[(t, 'acc')] = acc
                    co = 0
                    for (j, off, m) in xe_pieces(t):
                        xt = xe_bufs[j]
                        c = 0
                        while c < m:
                            pp = min(2, m - c)
                            f1g = f1pool.tile([128, 2, ROWE], fp32,
                                              name="f1g", tag="f1g")
                            for jj in range(pp):
                                nc.tensor.matmul(
                                    out=f1g[:, jj, 0:F],
                                    lhsT=xt[:, (off + c + jj) * 128:
                                            (off + c + jj + 1) * 128],
                                    rhs=w1_sb[:], start=True, stop=True)
                            nc.vector.tensor_tensor(
                                out=ge[:, co + c:co + c + pp, :]
                                .rearrange("p c (h d) -> p c h d", h=H),
                                in0=f1g[:, :pp, 0:F]
                                .rearrange("p c (h d) -> p c h d", h=H),
                                in1=ex[:, co + c:co + c + pp, :]
                                .unsqueeze(3).to_broadcast([128, pp, H, D]),
                                op=OP.mult)
                            for jj in range(pp):
                                nc.tensor.matmul(
                                    out=acc[:], lhsT=eye_sb[:],
                                    rhs=ge[:, co + c + jj, :],
                                    start=(co + c + jj == 0),
                                    stop=(co + c + jj == nch - 1))
                            c += pp
                        co += m

                def stD(t):
                    ex, rden = st.pop(t)
                    acc = st.pop((t, 'acc'))
                    finish_tile(t, acc[:], rden, 0, out_fn)

                pipeline([(0, stA), (1, stB), (3, stD), (2, stC)])

            # ================= gather-based layer (layer 2) ==============

            gpools = (gApool, gMpool, gBpool)

            def run_layer_gather(T, lidx, out_fn):
                Tw = [T[W_OFF[w]:W_OFF[w] + WIN, :] for w in range(3)]
                nextc = [0, 0, 0]
                bufs = [{}, {}, {}]

                def ensure(w, upto_chunk):
                    while nextc[w] * GCH < min(upto_chunk, TC[w]):
                        j = nextc[w]
                        lo = j * GCH
                        cnt = min(GCH, TC[w] - lo)
                        gt = gpools[w].tile([128, GCH, ROWE], bf16,
                                            name=f"g{w}", tag=f"g{w}")
                        nc.gpsimd.dma_gather(
                            gt[:, :cnt, :], Tw[w],
                            ix_sb[w][:, 8 * lo:8 * (lo + cnt)],
                            128 * cnt, nidx_reg(128 * cnt), ROWE,
                            single_packet=False, queue_num=next_q())
                        bufs[w][j] = gt
                        nextc[w] += 1

                def pieces(w, t):
                    out = []
                    lo, n = sW[w][t], CW[w][t]
                    while n > 0:
                        j = lo // GCH
                        off = lo - j * GCH
                        m = min(n, GCH - off)
                        out.append((j, off, m))
                        lo += m
                        n -= m
                    return out

                st = {}

                # emit ALL gather calls upfront, ordered by first-consumer
                # tile; pool buffer reuse provides the backpressure that
                # keeps ~bufs calls in flight across the 4 SWDGE queues.
                order = []
                for w in range(3):
                    ncall = (TC[w] + GCH - 1) // GCH
                    for j in range(ncall):
                        fct = next(t for t in range(TPC)
                                   if sW[w][t] + CW[w][t] > j * GCH)
                        order.append((fct, w, j))
                order.sort()
                for (_, w, j) in order:
                    ensure(w, j * GCH + 1)

                def stB(t):
                    nch = NCH[t]
                    pcs = [(w, *pc) for w in range(3)
                           for pc in pieces(w, t)]

                    def lg_src(lg, nch=nch, t=t, pcs=pcs):
                        co = 0
                        for (w, j, off, m) in pcs:
                            gt = bufs[w][j]
                            nc.vector.tensor_tensor(
                                out=lg[:, co:co + m, :],
                                in0=gt[:, off:off + m, F:F + 3],
                                in1=er2_own[:, t * 3:t * 3 + 3]
                                .unsqueeze(1).to_broadcast([128, m, 3]),
                                op=OP.add)
                            co += m

                    st[t] = softmax_tile(t, lg_src, lidx)

                def stC(t):
                    nch = NCH[t]
                    ex, _ = st[t]
                    pcs = [(w, *pc) for w in range(3)
                           for pc in pieces(w, t)]
                    acc = apool.tile([128, F], fp32, name="acc", tag="acc")
                    st[(t, 'acc')] = acc
                    co = 0
                    first = True
                    for (w, j, off, m) in pcs:
                        gt = bufs[w][j]
                        fv = gt[:, off:off + m, 0:F].rearrange(
                            "p c (h d) -> p c h d", h=H)
                        nc.vector.tensor_tensor(
                            out=fv, in0=fv,
                            in1=ex[:, co:co + m, :]
                            .unsqueeze(3).to_broadcast([128, m, H, D]),
                            op=OP.mult)
                        for c in range(m):
                            nc.tensor.matmul(
                                out=acc[:], lhsT=eye_sb[:],
                                rhs=gt[:, off + c, 0:F],
                                start=first,
                                stop=(co + c == nch - 1))
                            first = False
                        co += m

                def stD(t):
                    ex, rden = st.pop(t)
                    acc = st.pop((t, 'acc'))
                    finish_tile(t, acc[:], rden, lidx, out_fn)

                pipeline([(0, stB), (2, stD), (1, stC)])

            # ---- layer 1 output: build T2 rows, AllGather at end ----
            def l1_out(t, hta, htb):
                f2p = rwpool.tile([128, ROWE], fp32, name="f2p", tag="prow")
                nc.tensor.matmul(out=f2p[:], lhsT=hta[:], rhs=w2a_sb[:],
                                 start=True, stop=False)
                nc.tensor.matmul(out=f2p[:], lhsT=htb[:], rhs=w2b_sb[:],
                                 start=False, stop=True)
                f2s = brpool.tile([128, ROWE], bf16, name="f2s", tag="f1s")
                nc.vector.tensor_copy(f2s[:], f2p[:])
                nc.scalar.activation(er2_own[:, t * 3:t * 3 + 3],
                                     f2p[:, F + 3:F + 6], AF.Copy)
                nc.sync.dma_start(f2sl[t * 128:(t + 1) * 128, :], f2s[:])
                if t == TPC - 1:
                    nc.sync.dma_start(f2sl[6250:6251, :], patch_sb[0:1, :])
                    nc.gpsimd.collective_compute(
                        "AllGather", mybir.AluOpType.bypass,
                        replica_groups=[list(range(NCORES))],
                        ins=[f2sl[:]], outs=[T2[:]])

            run_layer1(l1_out)

            # ---- layer 2 output: hs rows + hd, AllGather at end ----
            def l2_out(t, hta, htb):
                hsp = erpool.tile([128, 2 * CLS], fp32, name="hsp", tag="er")
                nc.tensor.matmul(out=hsp[:], lhsT=hta[:], rhs=wpa_sb[:],
                                 start=True, stop=False)
                nc.tensor.matmul(out=hsp[:], lhsT=htb[:], rhs=wpb_sb[:],
                                 start=False, stop=True)
                hss = brpool.tile([128, 128], bf16, name="hss", tag="f1s")
                nc.scalar.activation(hss[:, 0:CLS], hsp[:, 0:CLS], AF.Copy)
                nc.vector.tensor_tensor(
                    out=hd_own[:, t * CLS:(t + 1) * CLS],
                    in0=hsp[:, CLS:2 * CLS], in1=bp_sb[:], op=OP.add)
                nc.sync.dma_start(hssl[t * 128:(t + 1) * 128, :], hss[:])
                if t == TPC - 1:
                    nc.gpsimd.collective_compute(
                        "AllGather", mybir.AluOpType.bypass,
                        replica_groups=[list(range(NCORES))],
                        ins=[hssl[:]], outs=[HSD[:]])

            run_layer_gather(T2, 1, l2_out)

            # ---- score pass ----
            def run_score():
                Hw = [HSD[W_OFF[w]:W_OFF[w] + WIN, :] for w in range(3)]
                nextc = [0, 0, 0]
                bufs = [{}, {}, {}]

                def ensure_s(w, upto):
                    while nextc[w] * GCH < min(upto, TC[w]):
                        j = nextc[w]
                        lo = j * GCH
                        cnt = min(GCH, TC[w] - lo)
                        # reuse the layer-2 gather buffers (same tag), but
                        # viewed as [128, 2*GCH, 128] for 256B score rows
                        gt0 = gpools[w].tile([128, GCH, ROWE], bf16,
                                             name=f"g{w}", tag=f"g{w}")
                        gt = gt0[:].rearrange("p c (a e) -> p (c a) e", a=2)
                        nc.gpsimd.dma_gather(
                            gt[:, :cnt, :], Hw[w],
                            ix_sb[w][:, 8 * lo:8 * (lo + cnt)],
                            128 * cnt, nidx_reg(128 * cnt), 128,
                            single_packet=False, queue_num=next_q())
                        bufs[w][j] = gt
                        nextc[w] += 1

                def pieces_s(t):
                    out = []
                    for w in range(3):
                        lo, n = sW[w][t], CW[w][t]
                        while n > 0:
                            j = lo // GCH
                            off = lo - j * GCH
                            m = min(n, GCH - off)
                            out.append((w, j, off, m))
                            lo += m
                            n -= m
                    return out

                order = []
                for w in range(3):
                    ncall = (TC[w] + GCH - 1) // GCH
                    for j in range(ncall):
                        fct = next(t for t in range(TPC)
                                   if sW[w][t] + CW[w][t] > j * GCH)
                        order.append((fct, w, j))
                order.sort()
                for (_, w, j) in order:
                    ensure_s(w, j * GCH + 1)

                def stB(t):
                    nch = NCH[t]
                    sc = scpool.tile([128, NCHMAX, CLS], fp32, name="sc",
                                     tag="sc")
                    co = 0
                    for (w, j, off, m) in pieces_s(t):
                        gt = bufs[w][j]
                        nc.vector.tensor_tensor(
                            out=sc[:, co:co + m, :],
                            in0=gt[:, off:off + m, 0:CLS],
                            in1=hd_own[:, t * CLS:(t + 1) * CLS]
                            .unsqueeze(1).to_broadcast([128, m, CLS]),
                            op=OP.add)
                        co += m
                    out_v = score_out[sT[t] * 128:(sT[t] + nch) * 128, :] \
                        .rearrange("(p c) j -> p c j", p=128)
                    nc.sync.dma_start(out_v, sc[:, :nch, :])

                pipeline([(0, stB)])

            run_score()

    mybir.codegen_inst_isa_subclasses(nc)
    _cap_waits(nc, mybir)
    return nc


def _cap_waits(nc, mybir, lim=1):
    """Walrus embeds at most `lim` semaphore waits per HW instruction.
    Move excess waits onto same-engine NoOps inserted just before."""
    eng_map = {
        mybir.EngineType.PE: nc.tensor,
        mybir.EngineType.DVE: nc.vector,
        mybir.EngineType.Activation: nc.scalar,
        mybir.EngineType.Pool: nc.gpsimd,
        mybir.EngineType.SP: nc.sync,
    }
    scratch = nc.main_func.blocks[-1].instructions
    for bb in nc.main_func.blocks:
        out = []
        for ins in bb.instructions:
            si = ins.sync_info
            waits = list(si.on_wait) if si is not None and si.on_wait else []
            if len(waits) > lim:
                keep = waits[-lim:]
                excess = waits[:-lim]
                eng = eng_map.get(ins.engine)
                assert eng is not None, f"no engine for {ins}"
                while excess:
                    grp, excess = excess[:lim], excess[lim:]
                    eng.nop(hint="waitsplit", nofuse=True)
                    nop = scratch.pop()
                    nop.sync_info = mybir.SyncInfo(on_wait=grp, on_update=[])
                    out.append(nop)
                ins.sync_info = mybir.SyncInfo(
                    on_wait=keep, on_update=list(si.on_update or []))
            out.append(ins)
        bb.instructions[:] = out


# ======================================================================
# entry point
# ======================================================================

def kernel(src, dst, nfeats, efeats, W1, al1, ar1, b1, W2, al2, ar2, b2,
           Wp, bp, _collect=None):
    import sys
    if '/opt/trn_rl_repo' not in sys.path:
        sys.path.insert(0, '/opt/trn_rl_repo')
    from concourse.bass_utils import run_bass_kernel_spmd

    sig, in_maps, orig, sW_np, CW_np = _prep(
        src, dst, nfeats, W1, al1, ar1, b1, W2, al2, ar2, b2, Wp, bp)
    if sig not in _COMPILED:
        _COMPILED[sig] = _build_program(sig)
    nc = _COMPILED[sig]

    kw = dict(_collect or {})
    kw.pop("results", None)
    res = run_bass_kernel_spmd(nc, in_maps, list(range(NCORES)), **kw)
    if _collect is not None:
        _collect["results"] = res

    # assemble: device slot order is tile-major, then window A,M,B chunks
    NCHt = (CW_np[0] + CW_np[1] + CW_np[2])
    sT = np.zeros(TPC, np.int64)
    sT[1:] = np.cumsum(NCHt[:-1])
    out = np.zeros((E, CLS), np.float32)
    for k in range(NCORES):
        sc = np.asarray(res.results[k]["score_out"])
        for t in range(TPC):
            nch = int(NCHt[t])
            # device rows for tile t: sT[t]*128 + p*nch + c
            blk = sc[sT[t] * 128:(sT[t] + nch) * 128].reshape(128, nch, CLS)
            co = 0
            for w in range(3):
                lo, n = int(sW_np[w][t]), int(CW_np[w][t])
                # orig index layout: position (lo+c)*128 + p
                o = orig[w][k][lo * 128:(lo + n) * 128].reshape(n, 128)
                rows = blk[:, co:co + n].transpose(1, 0, 2)  # [n,128,CLS]
                m = o >= 0
                out[o[m]] = rows[m]
                co += n
    return out
